# revision 1
# baseline (speedup 1.0000x reference)
"""Multi-head attention (B=2, S=4096, D=512, H=8) on 8 trn2 NeuronCores.

Sharding: head-parallel. Core i computes head i for BOTH batches (work per
head is proportional to that batch's valid_len, so pairing each head with
both batches balances the skewed valid_lens across cores). Each core
applies its row-slice of Wo on device and returns a full-shape partial;
the host sums the 8 partials (the tensor-parallel all-reduce, done in the
gather step).

Device dataflow per core (matmuls in fp16: same 1 cy/row PE rate as bf16
but 10 mantissa bits -> 5.9e-4 end-to-end error vs 4.8e-3 for bf16 and
3.1e-4 for float32r which runs at half rate; PSUM accumulates fp32):
  - projections:  Q^T[64,S] = Wq_h^T X^T,  K^T[64,vlp] likewise,
                  V[vlp,64] natural (plus a ones column -> softmax denom)
  - attention, per 512-wide q-block, streaming over 128-wide k-chunks:
        scoresT[k,q] = (K^T chunk as lhsT)^T @ Q^T block     (PE)
        E = exp(scoresT * 1/sqrt(hd) + mask_bias[k])         (ACT)
        outU[65,q]  += Vtilde_chunk^T @ E                    (PE, accum)
    row 64 of outU is the softmax denominator (ones column of Vtilde).
  - per q-block: outU -> SBUF, PE-transpose denominators to [q,1] layout,
    reciprocal, Wo matmul (lhsT = outU q-chunk), normalize with
    tensor_scalar_mul, DMA out.

Inputs beyond vl are never touched: K/V are projected only up to
vlp = ceil(vl/128)*128 and the boundary chunk is masked via the exp bias.
"""

import math
import os
from contextlib import ExitStack

import ml_dtypes
import numpy as np

import concourse.bass as bass
import concourse.mybir as mybir
import concourse.tile as tile
from concourse import bacc
from concourse import bass_utils

F32 = mybir.dt.float32
F32R = mybir.dt.float32r
MM_DT = F32R  # dtype of all matmul operands
EXP = mybir.ActivationFunctionType.Exp
NEG = -1.0e6

N_CORES = 8

# Problem shape (hardcoded per harness contract).
B_, S_, D_, H_ = 2, 4096, 512, 8
HD_ = D_ // H_


def _ceil_div(a, b):
    return (a + b - 1) // b


def _blocks(total, width):
    """[(offset, size), ...] covering `total` in chunks of `width`."""
    out = []
    off = 0
    while off < total:
        out.append((off, min(width, total - off)))
        off += width
    return out


def build_kernel(nc, cfg):
    """Emit the per-core kernel IR. cfg keys: S, D, HD, vlps (tuple per
    batch, each a multiple of 128)."""
    S, D, HD = cfg["S"], cfg["D"], cfg["HD"]
    mdt = {"f32r": F32R, "bf16": mybir.dt.bfloat16, "f16": mybir.dt.float16,
           "f32": F32}[cfg.get("dt", "f32r")]
    edt = mybir.dt.bfloat16 if cfg.get("ev_bf16") else mdt
    vlps = cfg["vlps"]
    B = len(vlps)
    ND = D // 128          # d-chunks
    scale = 1.0 / math.sqrt(HD)
    nch = [v // 128 for v in vlps]       # k-chunks per batch
    chbase = [sum(nch[:b]) for b in range(B)]   # chunk offset into mask/vbuf
    nch_tot = sum(nch)
    QB = 512                              # q-block width
    nqb = _ceil_div(S, QB)

    # ---- DRAM I/O ----
    qT = nc.dram_tensor("qT", [B, D, S], mdt, kind="ExternalInput").ap()
    kTs = [
        nc.dram_tensor(f"kT{b}", [D, vlps[b]], mdt, kind="ExternalInput").ap()
        for b in range(B)
    ]
    vTs = [
        nc.dram_tensor(f"vT{b}", [D, vlps[b]], mdt, kind="ExternalInput").ap()
        for b in range(B)
    ]
    wq = nc.dram_tensor("wq", [D, HD], mdt, kind="ExternalInput").ap()
    wk = nc.dram_tensor("wk", [D, HD], mdt, kind="ExternalInput").ap()
    wv = nc.dram_tensor("wv", [D, HD], mdt, kind="ExternalInput").ap()
    wo = nc.dram_tensor("wo", [HD, D], mdt, kind="ExternalInput").ap()
    mask = nc.dram_tensor("mask", [128, nch_tot], F32, kind="ExternalInput").ap()
    out = nc.dram_tensor("out", [B, S, D], F32, kind="ExternalOutput").ap()

    with tile.TileContext(nc) as tc, ExitStack() as ctx:
        consts = ctx.enter_context(tc.tile_pool(name="consts", bufs=1))
        xt = ctx.enter_context(tc.tile_pool(name="xt", bufs=2 * ND + 2))
        qkv = ctx.enter_context(tc.tile_pool(name="qkv", bufs=1))
        epool = ctx.enter_context(tc.tile_pool(name="e", bufs=4))
        ousb = ctx.enter_context(tc.tile_pool(name="ousb", bufs=3))
        stage = ctx.enter_context(tc.tile_pool(name="stage", bufs=3))
        small = ctx.enter_context(tc.tile_pool(name="small", bufs=2))
        ps_mm = ctx.enter_context(tc.tile_pool(name="ps_mm", bufs=2, space="PSUM"))
        ps_sc = ctx.enter_context(tc.tile_pool(name="ps_sc", bufs=4, space="PSUM"))
        ps_ou = ctx.enter_context(tc.tile_pool(name="ps_ou", bufs=2, space="PSUM"))

        # ---- constants ----
        wq_sb = consts.tile([128, ND, HD], mdt)
        wk_sb = consts.tile([128, ND, HD], mdt)
        wv_sb = consts.tile([128, ND, HD], mdt)
        for w_sb, w_ap in ((wq_sb, wq), (wk_sb, wk), (wv_sb, wv)):
            nc.sync.dma_start(out=w_sb, in_=w_ap.rearrange("(c p) h -> p c h", p=128))
        wo_sb = consts.tile([HD, D], mdt)
        nc.sync.dma_start(out=wo_sb, in_=wo)
        mask_sb = consts.tile([128, nch_tot], F32)
        nc.sync.dma_start(out=mask_sb, in_=mask)
        # unit2: [HD+1, 2] with row HD ones; extracts the denominator row of
        # outU as a [q, 2] column pair via one tiny matmul per q-chunk.
        unit2_f32 = consts.tile([HD + 1, 2], F32)
        nc.vector.memset(unit2_f32, 0.0)
        nc.vector.memset(unit2_f32[HD : HD + 1, :], 1.0)
        unit2 = consts.tile([HD + 1, 2], mdt)
        nc.vector.tensor_copy(unit2, unit2_f32)

        ones_stage = consts.tile([128, nch_tot, 1], F32)
        nc.vector.memset(ones_stage, 1.0)

        # ---- phase A: projections ----
        def load_xt_tiles(src_ap, soff, swidth):
            tiles = []
            for dc in range(ND):
                t = xt.tile([128, QB], mdt, tag="xt")
                nc.sync.dma_start(
                    out=t[:, :swidth],
                    in_=src_ap[dc * 128 : (dc + 1) * 128, soff : soff + swidth],
                )
                tiles.append(t)
            return tiles

        def emit():
          # persistent projected tensors: b0 rows 0:64, b1 rows 64:128
          qT_sb = qkv.tile([64 * B, S], mdt)
          kT_sb = qkv.tile([64 * B, max(vlps)], mdt)
          # V with appended ones column, per k-chunk: [128, chunk, HD+1]
          vbuf = qkv.tile([128, nch_tot, HD + 1], edt)
          nc.vector.tensor_copy(vbuf[:, :, HD : HD + 1], ones_stage)

          # ---- phase B: attention + Wo (called per batch so the
          # next batch's projection DMA streams underneath) ----
          def phase_b(b):
              r0 = b * 64
              recip = small.tile([128, 4 * nqb], F32)
              for qb in range(nqb):
                  qoff = qb * QB
                  qw = min(QB, S - qoff)
                  nq128 = qw // 128
                  ou = ps_ou.tile([HD + 1, QB], F32)
                  for kc in range(nch[b]):
                      ssc = ps_sc.tile([128, QB], F32)
                      nc.tensor.matmul(
                          ssc[:, :qw],
                          kT_sb[r0 : r0 + 64, kc * 128 : (kc + 1) * 128],
                          qT_sb[r0 : r0 + 64, qoff : qoff + qw],
                          start=True,
                          stop=True,
                      )
                      e = epool.tile([128, QB], edt)
                      if cfg.get("exp_on_dve"):
                          nc.vector.tensor_copy(e[:, :qw], ssc[:, :qw])
                      else:
                          nc.scalar.activation(
                              e[:, :qw],
                              ssc[:, :qw],
                              EXP,
                              bias=mask_sb[:, chbase[b] + kc : chbase[b] + kc + 1],
                              scale=scale,
                          )
                      nc.tensor.matmul(
                          ou[:, :qw],
                          vbuf[:, chbase[b] + kc, :],
                          e[:, :qw],
                          start=(kc == 0),
                          stop=(kc == nch[b] - 1),
                      )
                  ou_sb = ousb.tile([HD + 1, QB], mdt)
                  nc.vector.tensor_copy(ou_sb[:, :qw], ou[:, :qw])
                  st = stage.tile([128, QB // 128, D], F32)
                  for qi in range(nq128):
                      # denominator row -> [q, 2] column pair, then reciprocal
                      dps = ps_mm.tile([128, 2], F32, tag="mm")
                      nc.tensor.matmul(
                          dps,
                          ou_sb[0 : HD + 1, qi * 128 : (qi + 1) * 128],
                          unit2,
                          start=True,
                          stop=True,
                      )
                      col = qb * 4 + qi
                      nc.vector.reciprocal(recip[:, col : col + 1], dps[:, 0:1])
                      wps = ps_mm.tile([128, D], F32, tag="mm")
                      nc.tensor.matmul(
                          wps,
                          ou_sb[0:HD, qi * 128 : (qi + 1) * 128],
                          wo_sb,
                          start=True,
                          stop=True,
                      )
                      nc.vector.tensor_scalar_mul(
                          st[:, qi, :], wps, recip[:, col : col + 1]
                      )
                  out_eng = nc.gpsimd if cfg.get("swdge_out") else nc.sync
                  if cfg.get("no_out_dma"):
                      out_eng.dma_start(
                          out=out[b, qoff : qoff + 128, 0:1].rearrange(
                              "(q p) n -> p q n", p=128
                          ),
                          in_=st[:, :1, :1],
                      )
                  else:
                      out_eng.dma_start(
                          out=out[b, qoff : qoff + qw, :].rearrange(
                              "(q p) n -> p q n", p=128
                          ),
                          in_=st[:, :nq128, :],
                      )


          if True:
            for b in range(B):
              r0 = b * 64
              # Q^T  [64, S]
              for soff, sw in _blocks(S, QB):
                  tiles = load_xt_tiles(qT[b], soff, sw)
                  ps = ps_mm.tile([64, QB], F32, tag="mm")
                  for dc in range(ND):
                      nc.tensor.matmul(
                          ps[:, :sw],
                          wq_sb[:, dc, :],
                          tiles[dc][:, :sw],
                          start=(dc == 0),
                          stop=(dc == ND - 1),
                      )
                  nc.vector.tensor_copy(qT_sb[r0 : r0 + 64, soff : soff + sw], ps[:, :sw])
              # K^T  [64, vlp]
              for soff, sw in _blocks(vlps[b], QB):
                  tiles = load_xt_tiles(kTs[b], soff, sw)
                  ps = ps_mm.tile([64, QB], F32, tag="mm")
                  for dc in range(ND):
                      nc.tensor.matmul(
                          ps[:, :sw],
                          wk_sb[:, dc, :],
                          tiles[dc][:, :sw],
                          start=(dc == 0),
                          stop=(dc == ND - 1),
                      )
                  nc.vector.tensor_copy(kT_sb[r0 : r0 + 64, soff : soff + sw], ps[:, :sw])
              # V natural [vlp, HD] per 128-chunk
              for soff, sw in _blocks(vlps[b], QB):
                  tiles = load_xt_tiles(vTs[b], soff, sw)
                  for sub in range(sw // 128):
                      ps = ps_mm.tile([128, HD], F32, tag="mm")
                      for dc in range(ND):
                          nc.tensor.matmul(
                              ps,
                              tiles[dc][:, sub * 128 : (sub + 1) * 128],
                              wv_sb[:, dc, :],
                              start=(dc == 0),
                              stop=(dc == ND - 1),
                          )
                      kc = chbase[b] + (soff + sub * 128) // 128
                      nc.vector.tensor_copy(vbuf[:, kc, 0:HD], ps)
              if not cfg.get("split_phases", True):
                  phase_b(b)
          if cfg.get("split_phases", True):
              for b in range(B):
                  phase_b(b)

        for _ in range(cfg.get("repeat", 1)):
            emit()

    nc.compile()
    return nc


def prepare_in_maps(queries, keys, values, vls, Wq, Wk, Wv, Wo, vlps,
                    np_dt=np.float32):
    """Host-side layout prep: transposes, trims, per-core weight slices, mask."""
    HD = HD_
    queries, keys, values = (x.astype(np_dt) for x in (queries, keys, values))
    Wq, Wk, Wv, Wo = (x.astype(np_dt) for x in (Wq, Wk, Wv, Wo))
    qT = np.ascontiguousarray(queries.transpose(0, 2, 1))          # [B, D, S]
    kT = [np.ascontiguousarray(keys[b].T[:, : vlps[b]]) for b in range(B_)]
    vT = [np.ascontiguousarray(values[b].T[:, : vlps[b]]) for b in range(B_)]
    nch = [v // 128 for v in vlps]
    mask_np = np.zeros((128, sum(nch)), dtype=np.float32)
    cb = 0
    for b in range(B_):
        idx = np.arange(vlps[b]).reshape(nch[b], 128).T   # [128, nch]
        mask_np[:, cb : cb + nch[b]] = np.where(idx < vls[b], 0.0, NEG)
        cb += nch[b]

    in_maps = []
    for c in range(N_CORES):
        h0 = c * HD
        m = {
            "qT": qT,
            "wq": np.ascontiguousarray(Wq[:, h0 : h0 + HD]),
            "wk": np.ascontiguousarray(Wk[:, h0 : h0 + HD]),
            "wv": np.ascontiguousarray(Wv[:, h0 : h0 + HD]),
            "wo": np.ascontiguousarray(Wo[h0 : h0 + HD, :]),
            "mask": mask_np,
        }
        for b in range(B_):
            m[f"kT{b}"] = kT[b]
            m[f"vT{b}"] = vT[b]
        in_maps.append(m)
    return in_maps


_NC_CACHE = {}

# Matmul dtype for the graded kernel: "f32r" (accurate, ~2cy/row on HW) or
# "bf16" (fast). Overridable via env for experiments.
DEFAULT_DT = os.environ.get("KERNEL_DT", "f16")


def _get_nc(cfg_key):
    if cfg_key not in _NC_CACHE:
        S, D, HD, vlps, dt = cfg_key
        nc = bacc.Bacc(
            "TRN2",
            target_bir_lowering=False,
            debug=False,
            enable_asserts=False,
            num_devices=N_CORES,
        )
        build_kernel(nc, {"S": S, "D": D, "HD": HD, "vlps": vlps, "dt": dt})
        _NC_CACHE[cfg_key] = nc
    return _NC_CACHE[cfg_key]


LAST_RESULT = None  # BassKernelResults of the most recent kernel() call
LAST_IN_MAPS = None


def kernel(queries, keys, values, valid_lens, Wq, Wk, Wv, Wo, _trace=False):
    global LAST_RESULT, LAST_IN_MAPS
    queries = np.ascontiguousarray(np.asarray(queries, dtype=np.float32))
    keys = np.ascontiguousarray(np.asarray(keys, dtype=np.float32))
    values = np.ascontiguousarray(np.asarray(values, dtype=np.float32))
    Wq = np.ascontiguousarray(np.asarray(Wq, dtype=np.float32))
    Wk = np.ascontiguousarray(np.asarray(Wk, dtype=np.float32))
    Wv = np.ascontiguousarray(np.asarray(Wv, dtype=np.float32))
    Wo = np.ascontiguousarray(np.asarray(Wo, dtype=np.float32))
    vls = [int(v) for v in np.asarray(valid_lens).reshape(-1)]

    Bq, S, D = queries.shape
    assert (Bq, S, D) == (B_, S_, D_), (Bq, S, D)
    HD = HD_
    vlps = tuple(min(S, _ceil_div(max(v, 1), 128) * 128) for v in vls)

    dt = DEFAULT_DT
    nc = _get_nc((S, D, HD, vlps, dt))
    np_dt = {"bf16": ml_dtypes.bfloat16, "f16": np.float16}.get(dt, np.float32)
    in_maps = prepare_in_maps(
        queries, keys, values, vls, Wq, Wk, Wv, Wo, vlps, np_dt=np_dt
    )
    LAST_IN_MAPS = in_maps
    LAST_RESULT = bass_utils.run_bass_kernel_spmd(
        nc, in_maps, core_ids=list(range(N_CORES)), trace=_trace
    )
    acc = np.zeros((B_, S, D), dtype=np.float32)
    for r in LAST_RESULT.results:
        acc += r["out"]
    return acc



# revision 33
# speedup vs baseline: 11.5848x; 11.5848x over previous
"""Multi-head attention (B=2, S=4096, D=512, H=8) on 8 trn2 NeuronCores.

Sharding: head-parallel. Core i computes head i for BOTH batches (work per
head is proportional to that batch's valid_len, so pairing each head with
both batches balances the skewed valid_lens across cores). Each core
applies its row-slice of Wo on device and returns a full-shape fp16
partial (halves the output HBM traffic vs fp32; the summed fp32 result
keeps ~6e-4 relative accuracy); the host sums the 8 partials in fp32 (the
tensor-parallel all-reduce, done in the gather step).

Device dataflow per core (matmuls in fp16: same 1 cy/row PE rate as bf16
but 10 mantissa bits -> 5.9e-4 end-to-end error vs 4.8e-3 for bf16 and
3.1e-4 for float32r which runs at half rate; PSUM accumulates fp32):
  - projections:  Q^T[64,S] = Wq_h^T X^T,  K^T[64,vlp] likewise,
                  V[vlp,64] natural (plus a ones column -> softmax denom)
  - attention, per 512-wide q-block, streaming over 128-wide k-chunks:
        scoresT[k,q] = (K^T chunk as lhsT)^T @ Q^T block     (PE)
        E = exp(scoresT * 1/sqrt(hd) + mask_bias[k])         (ACT)
        outU[65,q]  += Vtilde_chunk^T @ E                    (PE, accum)
    row 64 of outU is the softmax denominator (ones column of Vtilde).
  - per q-block: outU -> SBUF, PE-transpose denominators to [q,1] layout,
    reciprocal, Wo matmul (lhsT = outU q-chunk), normalize with
    tensor_scalar_mul, DMA out.

Inputs beyond vl are never touched: K/V are projected only up to
vlp = ceil(vl/128)*128 and the boundary chunk is masked via the exp bias.
"""

import math
import os
from contextlib import ExitStack

import ml_dtypes
import numpy as np

import concourse.bass as bass
import concourse.mybir as mybir
import concourse.tile as tile
from concourse import bacc
from concourse import bass_utils

F32 = mybir.dt.float32
F32R = mybir.dt.float32r
MM_DT = F32R  # dtype of all matmul operands
EXP = mybir.ActivationFunctionType.Exp
NEG = -1.0e6

N_CORES = 8

# Problem shape (hardcoded per harness contract).
B_, S_, D_, H_ = 2, 4096, 512, 8
HD_ = D_ // H_


def _ceil_div(a, b):
    return (a + b - 1) // b


def _blocks(total, width):
    """[(offset, size), ...] covering `total` in chunks of `width`."""
    out = []
    off = 0
    while off < total:
        out.append((off, min(width, total - off)))
        off += width
    return out


def build_kernel(nc, cfg):
    """Emit the per-core kernel IR. cfg keys: S, D, HD, vlps (tuple per
    batch, each a multiple of 128)."""
    S, D, HD = cfg["S"], cfg["D"], cfg["HD"]
    mdt = {"f32r": F32R, "bf16": mybir.dt.bfloat16, "f16": mybir.dt.float16,
           "f32": F32}[cfg.get("dt", "f32r")]
    edt = mybir.dt.bfloat16 if cfg.get("ev_bf16") else mdt
    vlps = cfg["vlps"]
    B = len(vlps)
    ND = D // 128          # d-chunks
    scale = 1.0 / math.sqrt(HD)
    nch = [v // 128 for v in vlps]       # k-chunks per batch
    chbase = [sum(nch[:b]) for b in range(B)]   # chunk offset into mask/vbuf
    nch_tot = sum(nch)
    QB = 512                              # q-block width
    nqb = _ceil_div(S, QB)

    # ---- DRAM I/O ----
    qT = nc.dram_tensor("qT", [B, D, S], mdt, kind="ExternalInput").ap()
    kTs = [
        nc.dram_tensor(f"kT{b}", [D, vlps[b]], mdt, kind="ExternalInput").ap()
        for b in range(B)
    ]
    vTs = [
        nc.dram_tensor(f"vT{b}", [D, vlps[b]], mdt, kind="ExternalInput").ap()
        for b in range(B)
    ]
    wq = nc.dram_tensor("wq", [D, HD], mdt, kind="ExternalInput").ap()
    wk = nc.dram_tensor("wk", [D, HD], mdt, kind="ExternalInput").ap()
    wv = nc.dram_tensor("wv", [D, HD], mdt, kind="ExternalInput").ap()
    wo = nc.dram_tensor("wo", [HD, D], mdt, kind="ExternalInput").ap()
    mask = nc.dram_tensor("mask", [128, nch_tot], F32, kind="ExternalInput").ap()
    out = nc.dram_tensor("out", [B, S, D], mybir.dt.float16, kind="ExternalOutput").ap()

    with tile.TileContext(nc) as tc, ExitStack() as ctx:
        consts = ctx.enter_context(tc.tile_pool(name="consts", bufs=1))
        xt = ctx.enter_context(tc.tile_pool(name="xt", bufs=2 * ND + 2))
        qkv = ctx.enter_context(tc.tile_pool(name="qkv", bufs=1))
        epool = ctx.enter_context(tc.tile_pool(name="e", bufs=4))
        ousb = ctx.enter_context(tc.tile_pool(name="ousb", bufs=3))
        stage = ctx.enter_context(tc.tile_pool(name="stage", bufs=3))
        small = ctx.enter_context(tc.tile_pool(name="small", bufs=2))
        ps_mm = ctx.enter_context(tc.tile_pool(name="ps_mm", bufs=2, space="PSUM"))
        ps_sc = ctx.enter_context(tc.tile_pool(name="ps_sc", bufs=4, space="PSUM"))
        ps_ou = ctx.enter_context(tc.tile_pool(name="ps_ou", bufs=2, space="PSUM"))

        # ---- constants ----
        wq_sb = consts.tile([128, ND, HD], mdt)
        wk_sb = consts.tile([128, ND, HD], mdt)
        wv_sb = consts.tile([128, ND, HD], mdt)
        for w_sb, w_ap in ((wq_sb, wq), (wk_sb, wk), (wv_sb, wv)):
            nc.sync.dma_start(out=w_sb, in_=w_ap.rearrange("(c p) h -> p c h", p=128))
        wo_sb = consts.tile([HD, D], mdt)
        nc.sync.dma_start(out=wo_sb, in_=wo)
        mask_sb = consts.tile([128, nch_tot], F32)
        nc.sync.dma_start(out=mask_sb, in_=mask)
        # unit2: [HD+1, 2] with row HD ones; extracts the denominator row of
        # outU as a [q, 2] column pair via one tiny matmul per q-chunk.
        unit2_f32 = consts.tile([HD + 1, 2], F32)
        nc.vector.memset(unit2_f32, 0.0)
        nc.vector.memset(unit2_f32[HD : HD + 1, :], 1.0)
        unit2 = consts.tile([HD + 1, 2], mdt)
        nc.vector.tensor_copy(unit2, unit2_f32)

        ones_stage = consts.tile([128, nch_tot, 1], F32)
        nc.vector.memset(ones_stage, 1.0)

        # ---- phase A: projections ----
        def load_xt_tiles(src_ap, soff, swidth):
            tiles = []
            for dc in range(ND):
                t = xt.tile([128, QB], mdt, tag="xt")
                nc.sync.dma_start(
                    out=t[:, :swidth],
                    in_=src_ap[dc * 128 : (dc + 1) * 128, soff : soff + swidth],
                )
                tiles.append(t)
            return tiles

        def emit():
          # persistent projected tensors: b0 rows 0:64, b1 rows 64:128
          qT_sb = qkv.tile([64 * B, S], mdt)
          kT_sb = qkv.tile([64 * B, max(vlps)], mdt)
          # V with appended ones column, per k-chunk: [128, chunk, HD+1]
          vbuf = qkv.tile([128, nch_tot, HD + 1], edt)
          nc.vector.tensor_copy(vbuf[:, :, HD : HD + 1], ones_stage)

          # ---- phase B: attention + Wo (called per batch so the
          # next batch's projection DMA streams underneath) ----
          def phase_b(b):
              r0 = b * 64
              recip = small.tile([128, 4 * nqb], F32)
              for qb in range(nqb):
                  qoff = qb * QB
                  qw = min(QB, S - qoff)
                  nq128 = qw // 128
                  ou = ps_ou.tile([HD + 1, QB], F32)
                  for kc in range(nch[b]):
                      ssc = ps_sc.tile([128, QB], F32)
                      nc.tensor.matmul(
                          ssc[:, :qw],
                          kT_sb[r0 : r0 + 64, kc * 128 : (kc + 1) * 128],
                          qT_sb[r0 : r0 + 64, qoff : qoff + qw],
                          start=True,
                          stop=True,
                      )
                      e = epool.tile([128, QB], edt)
                      if cfg.get("exp_on_dve"):
                          nc.vector.tensor_copy(e[:, :qw], ssc[:, :qw])
                      else:
                          nc.scalar.activation(
                              e[:, :qw],
                              ssc[:, :qw],
                              EXP,
                              bias=mask_sb[:, chbase[b] + kc : chbase[b] + kc + 1],
                              scale=scale,
                          )
                      nc.tensor.matmul(
                          ou[:, :qw],
                          vbuf[:, chbase[b] + kc, :],
                          e[:, :qw],
                          start=(kc == 0),
                          stop=(kc == nch[b] - 1),
                      )
                  ou_sb = ousb.tile([HD + 1, QB], mdt)
                  nc.vector.tensor_copy(ou_sb[:, :qw], ou[:, :qw])
                  st = stage.tile([128, QB // 128, D], mybir.dt.float16)
                  for qi in range(nq128):
                      # denominator row -> [q, 2] column pair, then reciprocal
                      dps = ps_mm.tile([128, 2], F32, tag="mm")
                      nc.tensor.matmul(
                          dps,
                          ou_sb[0 : HD + 1, qi * 128 : (qi + 1) * 128],
                          unit2,
                          start=True,
                          stop=True,
                      )
                      col = qb * 4 + qi
                      nc.vector.reciprocal(recip[:, col : col + 1], dps[:, 0:1])
                      wps = ps_mm.tile([128, D], F32, tag="mm")
                      nc.tensor.matmul(
                          wps,
                          ou_sb[0:HD, qi * 128 : (qi + 1) * 128],
                          wo_sb,
                          start=True,
                          stop=True,
                      )
                      nc.vector.tensor_scalar_mul(
                          st[:, qi, :], wps, recip[:, col : col + 1]
                      )
                  out_eng = nc.gpsimd if cfg.get("swdge_out") else nc.sync
                  if cfg.get("no_out_dma"):
                      out_eng.dma_start(
                          out=out[b, qoff : qoff + 128, 0:1].rearrange(
                              "(q p) n -> p q n", p=128
                          ),
                          in_=st[:, :1, :1],
                      )
                  else:
                      out_eng.dma_start(
                          out=out[b, qoff : qoff + qw, :].rearrange(
                              "(q p) n -> p q n", p=128
                          ),
                          in_=st[:, :nq128, :],
                      )


          if True:
            for b in range(B):
              r0 = b * 64
              # Q^T  [64, S]
              for soff, sw in _blocks(S, QB):
                  tiles = load_xt_tiles(qT[b], soff, sw)
                  ps = ps_mm.tile([64, QB], F32, tag="mm")
                  for dc in range(ND):
                      nc.tensor.matmul(
                          ps[:, :sw],
                          wq_sb[:, dc, :],
                          tiles[dc][:, :sw],
                          start=(dc == 0),
                          stop=(dc == ND - 1),
                      )
                  nc.vector.tensor_copy(qT_sb[r0 : r0 + 64, soff : soff + sw], ps[:, :sw])
              # K^T  [64, vlp]
              for soff, sw in _blocks(vlps[b], QB):
                  tiles = load_xt_tiles(kTs[b], soff, sw)
                  ps = ps_mm.tile([64, QB], F32, tag="mm")
                  for dc in range(ND):
                      nc.tensor.matmul(
                          ps[:, :sw],
                          wk_sb[:, dc, :],
                          tiles[dc][:, :sw],
                          start=(dc == 0),
                          stop=(dc == ND - 1),
                      )
                  nc.vector.tensor_copy(kT_sb[r0 : r0 + 64, soff : soff + sw], ps[:, :sw])
              # V natural [vlp, HD] per 128-chunk
              for soff, sw in _blocks(vlps[b], QB):
                  tiles = load_xt_tiles(vTs[b], soff, sw)
                  for sub in range(sw // 128):
                      ps = ps_mm.tile([128, HD], F32, tag="mm")
                      for dc in range(ND):
                          nc.tensor.matmul(
                              ps,
                              tiles[dc][:, sub * 128 : (sub + 1) * 128],
                              wv_sb[:, dc, :],
                              start=(dc == 0),
                              stop=(dc == ND - 1),
                          )
                      kc = chbase[b] + (soff + sub * 128) // 128
                      nc.vector.tensor_copy(vbuf[:, kc, 0:HD], ps)
              if not cfg.get("split_phases", True):
                  phase_b(b)
          if cfg.get("split_phases", True):
              for b in range(B):
                  phase_b(b)

        for _ in range(cfg.get("repeat", 1)):
            emit()

    nc.compile()
    return nc


def prepare_in_maps(queries, keys, values, vls, Wq, Wk, Wv, Wo, vlps,
                    np_dt=np.float32):
    """Host-side layout prep: transposes, trims, per-core weight slices, mask."""
    HD = HD_
    queries, keys, values = (x.astype(np_dt) for x in (queries, keys, values))
    Wq, Wk, Wv, Wo = (x.astype(np_dt) for x in (Wq, Wk, Wv, Wo))
    qT = np.ascontiguousarray(queries.transpose(0, 2, 1))          # [B, D, S]
    kT = [np.ascontiguousarray(keys[b].T[:, : vlps[b]]) for b in range(B_)]
    vT = [np.ascontiguousarray(values[b].T[:, : vlps[b]]) for b in range(B_)]
    nch = [v // 128 for v in vlps]
    mask_np = np.zeros((128, sum(nch)), dtype=np.float32)
    cb = 0
    for b in range(B_):
        idx = np.arange(vlps[b]).reshape(nch[b], 128).T   # [128, nch]
        mask_np[:, cb : cb + nch[b]] = np.where(idx < vls[b], 0.0, NEG)
        cb += nch[b]

    in_maps = []
    for c in range(N_CORES):
        h0 = c * HD
        m = {
            "qT": qT,
            "wq": np.ascontiguousarray(Wq[:, h0 : h0 + HD]),
            "wk": np.ascontiguousarray(Wk[:, h0 : h0 + HD]),
            "wv": np.ascontiguousarray(Wv[:, h0 : h0 + HD]),
            "wo": np.ascontiguousarray(Wo[h0 : h0 + HD, :]),
            "mask": mask_np,
        }
        for b in range(B_):
            m[f"kT{b}"] = kT[b]
            m[f"vT{b}"] = vT[b]
        in_maps.append(m)
    return in_maps


_NC_CACHE = {}

# Matmul dtype for the graded kernel: "f32r" (accurate, ~2cy/row on HW) or
# "bf16" (fast). Overridable via env for experiments.
DEFAULT_DT = os.environ.get("KERNEL_DT", "f16")


def _get_nc(cfg_key):
    if cfg_key not in _NC_CACHE:
        S, D, HD, vlps, dt = cfg_key
        nc = bacc.Bacc(
            "TRN2",
            target_bir_lowering=False,
            debug=False,
            enable_asserts=False,
            num_devices=N_CORES,
        )
        build_kernel(nc, {"S": S, "D": D, "HD": HD, "vlps": vlps, "dt": dt})
        _NC_CACHE[cfg_key] = nc
    return _NC_CACHE[cfg_key]


LAST_RESULT = None  # BassKernelResults of the most recent kernel() call
LAST_IN_MAPS = None


def kernel(queries, keys, values, valid_lens, Wq, Wk, Wv, Wo, _trace=False):
    global LAST_RESULT, LAST_IN_MAPS
    queries = np.ascontiguousarray(np.asarray(queries, dtype=np.float32))
    keys = np.ascontiguousarray(np.asarray(keys, dtype=np.float32))
    values = np.ascontiguousarray(np.asarray(values, dtype=np.float32))
    Wq = np.ascontiguousarray(np.asarray(Wq, dtype=np.float32))
    Wk = np.ascontiguousarray(np.asarray(Wk, dtype=np.float32))
    Wv = np.ascontiguousarray(np.asarray(Wv, dtype=np.float32))
    Wo = np.ascontiguousarray(np.asarray(Wo, dtype=np.float32))
    vls = [int(v) for v in np.asarray(valid_lens).reshape(-1)]

    Bq, S, D = queries.shape
    assert (Bq, S, D) == (B_, S_, D_), (Bq, S, D)
    HD = HD_
    vlps = tuple(min(S, _ceil_div(max(v, 1), 128) * 128) for v in vls)

    dt = DEFAULT_DT
    nc = _get_nc((S, D, HD, vlps, dt))
    np_dt = {"bf16": ml_dtypes.bfloat16, "f16": np.float16}.get(dt, np.float32)
    in_maps = prepare_in_maps(
        queries, keys, values, vls, Wq, Wk, Wv, Wo, vlps, np_dt=np_dt
    )
    LAST_IN_MAPS = in_maps
    LAST_RESULT = bass_utils.run_bass_kernel_spmd(
        nc, in_maps, core_ids=list(range(N_CORES)), trace=_trace
    )
    acc = np.zeros((B_, S, D), dtype=np.float32)
    for r in LAST_RESULT.results:
        acc += r["out"].astype(np.float32)
    return acc

